# revision 17
# baseline (speedup 1.0000x reference)
"""TRN2 Bass kernel: out = inp @ weights + bias.

Shapes (hardcoded): inp [16384, 4096] f32, weights [4096, 8192] f32,
bias [8192] f32 -> out [16384, 8192] f32.

Strategy (measured ~1.81 ms/core steady-state vs the 1.748 ms bf16 PE
roofline; the 2.268 ms fp32r baseline lost ~0.5 ms to unoverlapped
per-matmul stationary loads):
  - 8 NeuronCores as a 4 (batch) x 2 (contraction K) grid.
    Core c = (bi, ki) computes the partial GEMM
      part[bi,ki] = inpT[ki*2048:(ki+1)*2048, bi*4096:(bi+1)*4096].T
                    @ weights[ki*2048:(ki+1)*2048, :]
    i.e. per-core M=4096, K=2048, N=8192. The host upcasts the two
    bf16 K-partials, sums them and adds bias in f32.
  - Operands are cast to bf16 on the host: full PE stream rate
    (1 col/cycle), half the HBM traffic of f32. Unlike fp32r, bf16
    matmuls are legalized into separate InstLdweights + non-self-
    loading InstMatmult, so the 128-cycle PE weight load overlaps the
    previous matmul's streaming (fp32r self-loading matmuls serialize
    it: +123 cycles per matmul on HW). Net rel err ~2.9e-3.
    fp8e4 DoubleRow was measured at exactly 2x bf16 rate (157 TF/s),
    so a 3-term split-precision fp8 scheme (1.5x work) cannot beat
    bf16 at the 2e-2 accuracy bar.
  - Inner loop: x panel [128, 16, 1024] cached in SBUF (bufs=2);
    for each (ks, ms) x-tile kept stationary in the PE array, 4
    matmuls stream 4 512-wide N chunks into 4 PSUM banks; bank groups
    ping-pong (4+4) so DVE PSUM evacuation overlaps the next group.
    Partials are written back as bf16 (halves output DMA).
  - The whole body sits in a For_i loop whose trip count is a runtime
    input tensor ("niter", normally 1; values_load must skip its
    runtime bounds check, which kills the NEFF on HW). Timing runs use
    larger counts so per-iteration device time comes out as a slope,
    cancelling the 60-130 ms (bimodal) axon dispatch overhead.
"""
import sys

sys.path.insert(0, "/opt/trn_rl_repo")

import numpy as np

B, F, C = 16384, 4096, 8192
P = 128
NB, NK = 4, 2               # grid: batch-splits x K-splits
MB_CORE = B // NB           # 4096  per-core M
KB_CORE = F // NK           # 2048  per-core K
KS = KB_CORE // P           # 16    K subtiles of 128
M_BLK = 1024                # SBUF-cached M panel width
M_BLKS = MB_CORE // M_BLK   # 4
M_SUBS = M_BLK // P         # 8
N_GRP = 2048                # columns per PSUM bank group (4 banks x 512)
N_GRPS = C // N_GRP         # 4
BANKS = N_GRP // 512        # 4

_compiled = None
_last_in_maps = None


def _build(compile_hw=True, max_iters=256, use_loop=True, out_bf16=True, ng_outer=False,
           m_blk=M_BLK, kxm_bufs=2, alt_copy=False, n_grp=N_GRP, fp8=False):
    import contextlib

    import concourse.mybir as mybir
    import concourse.tile as tile
    from concourse import bacc
    from concourse.bass_interp import get_hw_module

    nc = bacc.Bacc("TRN2", target_bir_lowering=False, debug=False, num_devices=8)

    in_dt = mybir.dt.float8e4 if fp8 else mybir.dt.bfloat16
    x_dram = nc.dram_tensor("x", [KB_CORE, MB_CORE], in_dt, kind="ExternalInput")
    w_dram = nc.dram_tensor("w", [KB_CORE, C], in_dt, kind="ExternalInput")
    n_dram = nc.dram_tensor("niter", [1, 1], mybir.dt.int32, kind="ExternalInput")
    out_dt = mybir.dt.bfloat16 if out_bf16 else mybir.dt.float32
    out_dram = nc.dram_tensor("out", [MB_CORE, C], out_dt, kind="ExternalOutput")

    x_ap = x_dram.ap().rearrange("(ko p) m -> p ko m", p=P)   # [128, KS, 4096]
    w_ap = w_dram.ap().rearrange("(ko p) n -> p ko n", p=P)   # [128, KS, 8192]
    out_ap = out_dram.ap()

    with tile.TileContext(nc) as tc:
        with tc.tile_pool(name="kxm", bufs=kxm_bufs) as kxm_pool, \
             tc.tile_pool(name="kxn", bufs=2) as kxn_pool, \
             tc.tile_pool(name="outp", bufs=6) as out_pool, \
             tc.tile_pool(name="misc", bufs=1) as misc_pool, \
             tc.tile_pool(name="ps", bufs=8, space="PSUM") as psum_pool:
            nt = misc_pool.tile([1, 1], mybir.dt.int32, tag="nt")
            nc.sync.dma_start(nt[:], n_dram.ap())
            if use_loop == "const":
                loop_cm = tc.For_i(0, max_iters)
            elif use_loop:
                niter = nc.values_load(nt[:], min_val=1, max_val=max_iters,
                                       skip_runtime_bounds_check=True)
                loop_cm = tc.For_i(0, niter)
            else:
                loop_cm = contextlib.nullcontext()
            with loop_cm:
                m_blks = MB_CORE // m_blk
                m_subs = m_blk // P
                n_grps = C // n_grp
                banks = n_grp // 512
                outer = range(n_grps) if ng_outer else range(m_blks)
                inner = range(m_blks) if ng_outer else range(n_grps)
                for o in outer:
                    if ng_outer:
                        ng = o
                        kxn = kxn_pool.tile([P, KS, n_grp], in_dt, tag="kxn")
                        nc.sync.dma_start(kxn[:], w_ap[:, :, ng * n_grp:(ng + 1) * n_grp])
                    else:
                        mb = o
                        kxm = kxm_pool.tile([P, KS, m_blk], in_dt, tag="kxm")
                        nc.sync.dma_start(kxm[:], x_ap[:, :, mb * m_blk:(mb + 1) * m_blk])
                    for i in inner:
                        if ng_outer:
                            mb = i
                            kxm = kxm_pool.tile([P, KS, m_blk], in_dt, tag="kxm")
                            nc.sync.dma_start(kxm[:], x_ap[:, :, mb * m_blk:(mb + 1) * m_blk])
                        else:
                            ng = i
                            kxn = kxn_pool.tile([P, KS, n_grp], in_dt, tag="kxn")
                            nc.sync.dma_start(kxn[:], w_ap[:, :, ng * n_grp:(ng + 1) * n_grp])
                        for ms in range(m_subs):
                                psums = [psum_pool.tile([P, 512], mybir.dt.float32,
                                                        tag="ps", name=f"ps{b}")
                                         for b in range(banks)]
                                if fp8:
                                    for ks2 in range(KS // 2):
                                        lhsT = kxm[:, 2 * ks2:2 * ks2 + 2, ms * P:(ms + 1) * P]
                                        for b in range(banks):
                                            nc.tensor.matmul(
                                                psums[b],
                                                lhsT,
                                                kxn[:, 2 * ks2:2 * ks2 + 2, b * 512:(b + 1) * 512],
                                                start=(ks2 == 0),
                                                stop=(ks2 == KS // 2 - 1),
                                                perf_mode=mybir.MatmulPerfMode.DoubleRow,
                                            )
                                else:
                                    for ks in range(KS):
                                        lhsT = kxm[:, ks, ms * P:(ms + 1) * P]
                                        for b in range(banks):
                                            nc.tensor.matmul(
                                                psums[b],
                                                lhsT,
                                                kxn[:, ks, b * 512:(b + 1) * 512],
                                                start=(ks == 0),
                                                stop=(ks == KS - 1),
                                            )
                                for b in range(banks):
                                    ot = out_pool.tile([P, 512], out_dt, tag="ot")
                                    if alt_copy and b % 2:
                                        nc.scalar.copy(ot[:], psums[b])
                                    else:
                                        nc.vector.tensor_copy(ot[:], psums[b])
                                    nc.sync.dma_start(
                                        out_ap[mb * m_blk + ms * P: mb * m_blk + (ms + 1) * P,
                                               ng * n_grp + b * 512: ng * n_grp + (b + 1) * 512],
                                        ot[:],
                                    )

    nc.compile()
    if compile_hw:
        nc.m = get_hw_module(nc.m)
    return nc


def _transpose(a: np.ndarray) -> np.ndarray:
    """Fast-ish single-core host transpose of a 2D fp32 array."""
    try:
        import torch

        return torch.from_numpy(np.ascontiguousarray(a)).t().contiguous().numpy()
    except ImportError:
        pass
    r, c = a.shape
    bs = 128
    out = np.empty((c, r), np.float32)
    v = a.reshape(r // bs, bs, c // bs, bs)
    o = out.reshape(c // bs, bs, r // bs, bs)
    np.copyto(o, v.transpose(2, 3, 0, 1))
    return out


def kernel(inp: np.ndarray, weights: np.ndarray, bias: np.ndarray) -> np.ndarray:
    global _compiled
    import ml_dtypes
    from concourse import bass_utils

    if _compiled is None:
        _compiled = _build()
    nc = _compiled

    inp = np.ascontiguousarray(inp, dtype=np.float32)
    weights = np.ascontiguousarray(weights, dtype=np.float32)
    inpT = _transpose(inp).astype(ml_dtypes.bfloat16)   # [F, B] bf16
    w16 = weights.astype(ml_dtypes.bfloat16)            # [F, C] bf16
    one = np.array([[1]], dtype=np.int32)

    in_maps = []
    for bi in range(NB):
        for ki in range(NK):
            x_c = np.ascontiguousarray(
                inpT[ki * KB_CORE:(ki + 1) * KB_CORE, bi * MB_CORE:(bi + 1) * MB_CORE]
            )
            w_c = np.ascontiguousarray(w16[ki * KB_CORE:(ki + 1) * KB_CORE, :])
            in_maps.append({"x": x_c, "w": w_c, "niter": one})

    global _last_in_maps
    _last_in_maps = in_maps
    res = bass_utils.run_bass_kernel_spmd(nc, in_maps, list(range(NB * NK)))

    out = np.empty((B, C), np.float32)
    bias32 = bias.astype(np.float32, copy=False)
    for bi in range(NB):
        blk = out[bi * MB_CORE:(bi + 1) * MB_CORE]
        np.add(res.results[bi * NK]["out"].astype(np.float32),
               res.results[bi * NK + 1]["out"].astype(np.float32), out=blk)
        blk += bias32[None, :]
    return out

